# revision 17
# baseline (speedup 1.0000x reference)
"""Distributed causal multi-head attention for TRN2 (8 NeuronCores).

Sharding: tensor-parallel over heads (2 heads/core) for QKV projection and
attention; one AllToAll switches to sequence-sharding for the output
projection (each core emits 512 rows of the final output, stitched on
host).

v2 structure (vs the phase-separated v1):
  - QKV projection tiles are interleaved with attention tiles: proj(t)
    runs right before attn(t), so x^T transpose DMAs hide behind
    attention compute and the PE sees periodic full-array 128x128x512
    matmuls (keeps the HAM clock-gate at K=8/8 = 2.4 GHz; v1 ran the
    whole attention phase at 1.2 GHz).
  - A dummy-matmul warmup burst at t=0 un-throttles the PE before the
    first projection.
  - Score matmuls for the two local heads are contraction-64 and sit in
    distinct PE row groups (partitions 0:64 / 64:128, auto
    tile_position) with per-head double-buffered PSUM so they issue
    back-to-back and run concurrently in the array.
  - V tiles are transposed on the PE (transpose mode) instead of the
    DMA xbar; all x^T transposes go on the sync queue. The Scalar
    engine runs nothing but exp + reciprocal (it is the pacing engine:
    ~19M exp elements at ~150 G elem/s).
  - AV accumulates per head into a [65, 512] PSUM tile (row 64 = ones
    column of V_aug = softmax denominator). At tile end it is copied
    at once to SBUF to free the PSUM bank, then recip/broadcast/mul
    produce normalized vals off the critical path.

Layout discipline (contraction dim must sit on SBUF partitions):
  - x^T tiles   [d, s]    : xbar transpose-DMA from natural x
  - Q^T, K^T    [hk, s]   : direct result of projection matmuls (bf16)
  - V_aug       [skv, 65] : per skv-chunk, per head, bf16; col 64 = 1.0
  - S           [skv, sq] : PSUM f32; exp on ScalarE -> P bf16 in SBUF
  - vals^T      [hk, sq]  : bf16, A2A'd to sequence sharding
"""

import sys

sys.path.insert(0, "/opt/trn_rl_repo")

import ml_dtypes
import numpy as np

from concourse import bacc, bass, mybir, tile
from concourse.bass_utils import run_bass_kernel_spmd

S, D, H, K = 4096, 1024, 16, 64
NCORES = 8
HPC = H // NCORES          # heads per core (2)
HKC = HPC * K              # local head*dim columns (128)
SQ = S // NCORES           # seq rows owned per core (512)
SQT = 512                  # sq tile width in attention
NCH = S // 128             # total skv chunks (32)
F32 = mybir.dt.float32
BF16 = mybir.dt.bfloat16
EXP = mybir.ActivationFunctionType.Exp
BF16NP = ml_dtypes.bfloat16

_CACHE: dict = {}


def _build(causal: bool):
    nc = bacc.Bacc(
        "TRN2", target_bir_lowering=False, debug=False, num_devices=NCORES
    )
    cores = list(range(NCORES))

    x_full = nc.dram_tensor("x_full", [S, D], BF16, kind="ExternalInput")
    wq_c = nc.dram_tensor("wq_c", [D, HKC], BF16, kind="ExternalInput")
    wk_c = nc.dram_tensor("wk_c", [D, HKC], BF16, kind="ExternalInput")
    wv_c = nc.dram_tensor("wv_c", [D, HKC], BF16, kind="ExternalInput")
    wo_f = nc.dram_tensor("wo_f", [H * K, D], BF16, kind="ExternalInput")
    bq_c = nc.dram_tensor("bq_c", [HKC, 1], F32, kind="ExternalInput")
    bk_c = nc.dram_tensor("bk_c", [HKC, 1], F32, kind="ExternalInput")
    bv_c = nc.dram_tensor("bv_c", [HKC, 1], F32, kind="ExternalInput")
    bo_r = nc.dram_tensor("bo_r", [1, D], F32, kind="ExternalInput")
    masks = nc.dram_tensor("masks", [128, 128], F32, kind="ExternalInput")
    ident = nc.dram_tensor("ident", [128, 128], F32, kind="ExternalInput")
    out_t = nc.dram_tensor("out", [SQ, D], F32, kind="ExternalOutput")

    with tile.TileContext(nc) as tc:
        with tc.tile_pool(name="dram", bufs=1, space="DRAM") as dpool:
            a2a_in = dpool.tile([NCORES * HKC, SQ], BF16, name="a2a_in")
            a2a_out = dpool.tile([NCORES * HKC, SQ], BF16, name="a2a_out")

            with tc.tile_pool(name="persist", bufs=1) as pp:
                # ---- persistent SBUF ----
                dummy = pp.tile([128, SQT], BF16, name="dummy")
                nc.vector.memset(dummy, 0.5)
                wq_sb = pp.tile([128, 8, HKC], BF16, name="wq_sb")
                wk_sb = pp.tile([128, 8, HKC], BF16, name="wk_sb")
                wv_sb = pp.tile([128, 8, HKC], BF16, name="wv_sb")
                bq_sb = pp.tile([128, 1], F32, name="bq_sb")
                bk_sb = pp.tile([128, 1], F32, name="bk_sb")
                bv_sb = pp.tile([128, 1], F32, name="bv_sb")
                masks_sb = pp.tile([128, 128], F32, name="masks_sb")
                ident_sb = pp.tile([128, 128], F32, name="ident_sb")
                nbias = pp.tile([128, 1], F32, name="nbias")
                nc.vector.memset(nbias, -3.0)
                qT_sb = pp.tile([128, S], BF16, name="qT_sb")
                kT_sb = pp.tile([128, S], BF16, name="kT_sb")
                v_aug = pp.tile([128, NCH, HPC, K + 1], BF16, name="v_aug")
                nc.vector.memset(v_aug, 1.0)  # presets the ones columns
                vals_sb = pp.tile([128, S], BF16, name="vals_sb")
                wo_sb = pp.tile([128, 8, D], BF16, name="wo_sb")
                bo_sb = pp.tile([1, D], F32, name="bo_sb")
                bo_bc = pp.tile([128, D], F32, name="bo_bc")

                with tc.tile_pool(name="xtp", bufs=3) as xtp, tc.tile_pool(
                    name="vtp", bufs=2
                ) as vtp, tc.tile_pool(
                    name="pj", bufs=2, space="PSUM"
                ) as pj, tc.tile_pool(
                    name="pS", bufs=2, space="PSUM"
                ) as pSp, tc.tile_pool(
                    name="pV0", bufs=1, space="PSUM"
                ) as pVp0, tc.tile_pool(
                    name="pV1", bufs=1, space="PSUM"
                ) as pVp1, tc.tile_pool(name="pT", bufs=6) as pTp, tc.tile_pool(
                    name="sm", bufs=2
                ) as smp:
                    pVp = (pVp0, pVp1)

                    # PE warmup: un-throttle the HAM clock gate before the
                    # first projection (needs ~3.4us of sustained PE busy).
                    for _ in range(8):
                        ps = pj.tile([128, SQT], F32, name="ps")
                        nc.tensor.matmul(
                            ps, lhsT=dummy[:, 0:128], rhs=dummy,
                            start=True, stop=True,
                        )
                    # preload the exp activation table set off the critical
                    # path (first real exp would otherwise pay ~2.7us)
                    wexp = smp.tile([128, 1], BF16, name="wexp")
                    nc.scalar.activation(
                        out=wexp, in_=dummy[:, 0:1], func=EXP,
                        scale=0.125, bias=nbias,
                    )

                    xts = {}

                    def issue_xt(t, split=False):
                        xt = xtp.tile([128, 8, SQT], BF16, name="xt")
                        src = x_full.ap()[t * SQT : (t + 1) * SQT, :]
                        if split:
                            # split the first transpose by d-chunk across
                            # both hwdge queues: proj(0)'s dc=0..3 matmuls
                            # can start as soon as the scalar-queue half
                            # lands
                            nc.scalar.dma_start_transpose(
                                out=xt[:, 0:4, :],
                                in_=src[:, 0 : D // 2],
                            )
                            nc.sync.dma_start_transpose(
                                out=xt[:, 4:8, :],
                                in_=src[:, D // 2 : D],
                            )
                        else:
                            nc.sync.dma_start_transpose(out=xt, in_=src)
                        xts[t] = xt

                    # first weight + bias on sync ahead of everything so
                    # proj(0) is never input-starved
                    nc.sync.dma_start(
                        out=wq_sb,
                        in_=wq_c.ap().rearrange("(a p) h -> p a h", p=128),
                    )
                    nc.sync.dma_start(out=bq_sb, in_=bq_c.ap())
                    issue_xt(0, split=True)
                    for wsb, wdr in ((wk_sb, wk_c), (wv_sb, wv_c)):
                        nc.sync.dma_start(
                            out=wsb,
                            in_=wdr.ap().rearrange("(a p) h -> p a h", p=128),
                        )
                    for bsb, bdr in ((bk_sb, bk_c), (bv_sb, bv_c)):
                        nc.sync.dma_start(out=bsb, in_=bdr.ap())
                    nc.sync.dma_start(out=masks_sb, in_=masks.ap())
                    nc.sync.dma_start(out=ident_sb, in_=ident.ap())
                    issue_xt(1)

                    def proj_units(t):
                        """Projection for tile t as a list of small closures
                        sprinkled between attention chunk-steps: keeps
                        full-array matmuls flowing through the PE (HAM clock
                        gate stays warm) and hides proj work inside the
                        Scalar-paced attention pipeline."""
                        xt = xts.pop(t)
                        units = []

                        def mk_mms(wsb, box, dcs):
                            def go():
                                if box[0] is None:
                                    box[0] = pj.tile(
                                        [128, SQT], F32, name="ps"
                                    )
                                for dc in dcs:
                                    nc.tensor.matmul(
                                        box[0],
                                        lhsT=wsb[:, dc, :],
                                        rhs=xt[:, dc, :],
                                        start=(dc == 0),
                                        stop=(dc == 7),
                                    )
                            return go

                        def mk_bias(box, dst_sb, bias_sb):
                            def go():
                                nc.vector.tensor_scalar_add(
                                    out=dst_sb[:, t * SQT : (t + 1) * SQT],
                                    in0=box[0],
                                    scalar1=bias_sb,
                                )
                            return go

                        for which in range(3):
                            wsb = (wq_sb, wk_sb, wv_sb)[which]
                            box = [None]
                            for lo in (0, 4):
                                units.append(
                                    mk_mms(wsb, box, range(lo, lo + 4))
                                )
                            if which == 0:
                                units.append(mk_bias(box, qT_sb, bq_sb))
                            elif which == 1:
                                units.append(mk_bias(box, kT_sb, bk_sb))
                            else:
                                vb = [None]

                                def vbias(box=box, vb=vb):
                                    vb[0] = vtp.tile(
                                        [128, SQT], F32, name="vtt"
                                    )
                                    nc.vector.tensor_scalar_add(
                                        out=vb[0], in0=box[0], scalar1=bv_sb
                                    )
                                units.append(vbias)
                                # transpose V on the PE (v1 used the DMA
                                # xbar, which serialized on the hwdge queue)
                                pb = [None]

                                def mk_trans(js, vb=vb, pb=pb):
                                    def go():
                                        if pb[0] is None:
                                            pb[0] = pj.tile(
                                                [128, SQT], F32, name="ps"
                                            )
                                        for j in js:
                                            nc.tensor.transpose(
                                                pb[0][
                                                    :, j * 128 : (j + 1) * 128
                                                ],
                                                vb[0][
                                                    :, j * 128 : (j + 1) * 128
                                                ],
                                                ident_sb,
                                            )
                                    return go
                                units.append(mk_trans((0, 1)))
                                units.append(mk_trans((2, 3)))

                                def mk_copy(j, pb=pb):
                                    def go():
                                        for h in range(HPC):
                                            nc.vector.tensor_copy(
                                                out=v_aug[
                                                    :, t * 4 + j, h, 0:K
                                                ],
                                                in_=pb[0][
                                                    :,
                                                    j * 128 + h * K : j * 128
                                                    + (h + 1) * K,
                                                ],
                                            )
                                    return go
                                for j in range(4):
                                    units.append(mk_copy(j))
                        return units

                    def attn(t, pending):
                        nch = 4 * (t + 1) if causal else NCH
                        pv = {}
                        for h in range(HPC):
                            pv[h] = pVp[h].tile(
                                [K + 1, SQT], F32, name=f"pv{h}"
                            )
                        def s_and_exp(ch):
                            jm = ch - 4 * t
                            off = jm * 128 if (causal and 0 < jm < 4) else 0
                            # both heads share one PSUM tile (adjacent
                            # banks) so exp covers both in ONE ScalarE
                            # instruction (each instr pays ~293ns fixed)
                            pS = pSp.tile([128, HPC, SQT], F32, name="pS")
                            # paired score matmuls, back-to-back: head0 in
                            # PE rows 0:64, head1 in rows 64:128 -> they
                            # run concurrently in distinct row groups
                            for h in range(HPC):
                                hs = h * K
                                nc.tensor.matmul(
                                    pS[:, h, off:SQT],
                                    lhsT=kT_sb[
                                        hs : hs + K,
                                        ch * 128 : (ch + 1) * 128,
                                    ],
                                    rhs=qT_sb[
                                        hs : hs + K,
                                        t * SQT + off : (t + 1) * SQT,
                                    ],
                                    start=True,
                                    stop=True,
                                )
                            if causal and 0 <= jm < 4:
                                bo_ = jm * 128
                                for h in range(HPC):
                                    nc.vector.tensor_add(
                                        out=pS[:, h, bo_ : bo_ + 128],
                                        in0=pS[:, h, bo_ : bo_ + 128],
                                        in1=masks_sb,
                                    )
                            pT = pTp.tile([128, HPC, SQT], BF16, name="pT")
                            nc.scalar.activation(
                                out=pT[:, :, off:SQT],
                                in_=pS[:, :, off:SQT],
                                func=EXP,
                                scale=0.125,
                                bias=nbias,
                            )
                            return pT, off

                        def av(ch, pT, off):
                            for h in range(HPC):
                                nc.tensor.matmul(
                                    pv[h][:, off:SQT],
                                    lhsT=v_aug[:, ch, h, :],
                                    rhs=pT[:, h, off:SQT],
                                    start=(ch == 0),
                                    stop=(ch == nch - 1),
                                )

                        # software pipeline: scores/exp of step ch+1 are
                        # emitted BEFORE AV of step ch, so the in-order PE
                        # never sits directly behind the exp it feeds
                        prev = None
                        for ch in range(nch):
                            cur = (ch, *s_and_exp(ch))
                            if prev is not None:
                                av(*prev)
                            # sprinkle next tile's projection between
                            # chunk-steps (runs while exp(ch) is on ScalarE)
                            nun = len(pending)
                            if nun and ch < nch - 1:
                                k = max(1, -(-nun // (nch - 1 - ch)))
                                for _ in range(min(k, nun)):
                                    pending.pop(0)()
                            prev = cur
                        av(*prev)
                        while pending:
                            pending.pop(0)()
                        for h in range(HPC):
                            hs = h * K
                            # copy AV out of PSUM at once so the single pv
                            # bank frees for the next tile
                            cval = smp.tile([K + 1, SQT], F32, name="cval")
                            nc.vector.tensor_copy(out=cval, in_=pv[h])
                            recip = smp.tile([1, SQT], F32, name="recip")
                            # on Vector: a Scalar reciprocal would thrash
                            # the activation table set between exp/recip
                            # (~1.3us ACT_TABLE_LOAD per switch)
                            nc.vector.reciprocal(
                                out=recip, in_=cval[K : K + 1, :]
                            )
                            bcn = smp.tile([K, SQT], F32, name="bcn")
                            nc.gpsimd.partition_broadcast(bcn, recip)
                            nc.vector.tensor_mul(
                                out=vals_sb[
                                    hs : hs + K, t * SQT : (t + 1) * SQT
                                ],
                                in0=cval[0:K, :],
                                in1=bcn,
                            )
                            nc.sync.dma_start(
                                out=a2a_in[
                                    t * HKC + hs : t * HKC + hs + K, :
                                ],
                                in_=vals_sb[
                                    hs : hs + K, t * SQT : (t + 1) * SQT
                                ],
                            )

                    # proj(0) runs up front (nothing to interleave with);
                    # proj(t+1) is sprinkled through attn(t)'s chunk-steps
                    for u in proj_units(0):
                        u()
                    for t in range(8):
                        if t + 2 < 8:
                            issue_xt(t + 2)
                        if t == 1:
                            # big late loads, hidden behind attention
                            nc.sync.dma_start(
                                out=wo_sb,
                                in_=wo_f.ap().rearrange(
                                    "(a p) d -> p a d", p=128
                                ),
                            )
                            nc.sync.dma_start(out=bo_sb, in_=bo_r.ap())
                            nc.gpsimd.partition_broadcast(bo_bc, bo_sb)
                        pending = proj_units(t + 1) if t + 1 < 8 else []
                        attn(t, pending)

                nc.gpsimd.collective_compute(
                    "AllToAll",
                    mybir.AluOpType.bypass,
                    replica_groups=[cores],
                    ins=[a2a_in.opt()],
                    outs=[a2a_out.opt()],
                )

                # ---- output projection (sequence-sharded) ----
                with tc.tile_pool(name="op", bufs=1) as op, tc.tile_pool(
                    name="po", bufs=8, space="PSUM"
                ) as pop:
                    va_sb = op.tile([128, 8, SQT], BF16, name="va_sb")
                    a2a_or = a2a_out.rearrange("(a p) s -> p a s", p=128)
                    for hkc in range(8):
                        # per-chunk loads so the first matmuls start as
                        # soon as the first 128 hk rows land
                        nc.sync.dma_start(
                            out=va_sb[:, hkc, :], in_=a2a_or[:, hkc, :]
                        )
                    o_sb = op.tile([128, 4, D], F32, name="o_sb")
                    pos = {}
                    for m in range(4):
                        for dh in range(2):
                            pos[m, dh] = pop.tile([128, 512], F32, name="po")
                    for hkc in range(8):
                        for m in range(4):
                            for dh in range(2):
                                nc.tensor.matmul(
                                    pos[m, dh],
                                    lhsT=va_sb[:, hkc, m * 128 : (m + 1) * 128],
                                    rhs=wo_sb[:, hkc, dh * 512 : (dh + 1) * 512],
                                    start=(hkc == 0),
                                    stop=(hkc == 7),
                                )
                    for m in range(4):
                        for dh in range(2):
                            nc.vector.tensor_add(
                                out=o_sb[:, m, dh * 512 : (dh + 1) * 512],
                                in0=pos[m, dh],
                                in1=bo_bc[:, dh * 512 : (dh + 1) * 512],
                            )
                        nc.sync.dma_start(
                            out=out_t.ap()[m * 128 : (m + 1) * 128, :],
                            in_=o_sb[:, m, :],
                        )

    nc.compile()
    return nc


def _get_nc(causal: bool):
    if causal not in _CACHE:
        _CACHE[causal] = _build(causal)
    return _CACHE[causal]


def _make_in_maps(x, wq, bq, wk, bk, wv, bv, wo, bo):
    x = np.ascontiguousarray(
        np.asarray(x, np.float32).reshape(S, D).astype(BF16NP)
    )
    wqf = np.asarray(wq, np.float32).reshape(D, H * K).astype(BF16NP)
    wkf = np.asarray(wk, np.float32).reshape(D, H * K).astype(BF16NP)
    wvf = np.asarray(wv, np.float32).reshape(D, H * K).astype(BF16NP)
    wof = np.ascontiguousarray(
        np.asarray(wo, np.float32).reshape(H * K, D).astype(BF16NP)
    )
    bqf = np.asarray(bq, np.float32).reshape(H * K)
    bkf = np.asarray(bk, np.float32).reshape(H * K)
    bvf = np.asarray(bv, np.float32).reshape(H * K)
    bof = np.ascontiguousarray(np.asarray(bo, np.float32).reshape(1, D))

    p = np.arange(128)[:, None]
    c = np.arange(128)[None, :]
    mask_np = np.where(c >= p, 0.0, -1e9).astype(np.float32)
    ident_np = np.eye(128, dtype=np.float32)

    in_maps = []
    for core in range(NCORES):
        hk0 = core * HKC
        in_maps.append(
            {
                "x_full": x,
                "wq_c": np.ascontiguousarray(wqf[:, hk0 : hk0 + HKC]),
                "wk_c": np.ascontiguousarray(wkf[:, hk0 : hk0 + HKC]),
                "wv_c": np.ascontiguousarray(wvf[:, hk0 : hk0 + HKC]),
                "wo_f": wof,
                "bq_c": np.ascontiguousarray(
                    bqf[hk0 : hk0 + HKC].reshape(HKC, 1)
                ),
                "bk_c": np.ascontiguousarray(
                    bkf[hk0 : hk0 + HKC].reshape(HKC, 1)
                ),
                "bv_c": np.ascontiguousarray(
                    bvf[hk0 : hk0 + HKC].reshape(HKC, 1)
                ),
                "bo_r": bof,
                "masks": mask_np,
                "ident": ident_np,
            }
        )
    return in_maps


def _run(inputs: dict, trace: bool = False):
    causal = bool(int(np.asarray(inputs["is_causal"])))
    nc = _get_nc(causal)
    in_maps = _make_in_maps(
        inputs["x"], inputs["wq"], inputs["bq"], inputs["wk"], inputs["bk"],
        inputs["wv"], inputs["bv"], inputs["wo"], inputs["bo"],
    )
    res = run_bass_kernel_spmd(
        nc, in_maps, list(range(NCORES)), trace=trace
    )
    out = np.empty((1, S, D), np.float32)
    for core in range(NCORES):
        out[0, core * SQ : (core + 1) * SQ] = res.results[core]["out"]
    return out, res


def kernel(**inputs) -> np.ndarray:
    out, _ = _run(inputs, trace=False)
    return out


# revision 30
# speedup vs baseline: 1.0370x; 1.0370x over previous
"""Distributed causal multi-head attention for TRN2 (8 NeuronCores).

Sharding: tensor-parallel over heads (2 heads/core) for QKV projection and
attention; one AllToAll switches to sequence-sharding for the output
projection (each core emits 512 rows of the final output, stitched on
host).

v2 structure (vs the phase-separated v1):
  - QKV projection tiles are interleaved with attention tiles: proj(t)
    runs right before attn(t), so x^T transpose DMAs hide behind
    attention compute and the PE sees periodic full-array 128x128x512
    matmuls (keeps the HAM clock-gate at K=8/8 = 2.4 GHz; v1 ran the
    whole attention phase at 1.2 GHz).
  - A dummy-matmul warmup burst at t=0 un-throttles the PE before the
    first projection.
  - Score matmuls for the two local heads are contraction-64 and sit in
    distinct PE row groups (partitions 0:64 / 64:128, auto
    tile_position) with per-head double-buffered PSUM so they issue
    back-to-back and run concurrently in the array.
  - V tiles are transposed on the PE (transpose mode) instead of the
    DMA xbar; all x^T transposes go on the sync queue. The Scalar
    engine runs nothing but exp + reciprocal (it is the pacing engine:
    ~19M exp elements at ~150 G elem/s).
  - AV accumulates per head into a [65, 512] PSUM tile (row 64 = ones
    column of V_aug = softmax denominator). At tile end it is copied
    at once to SBUF to free the PSUM bank, then recip/broadcast/mul
    produce normalized vals off the critical path.

Layout discipline (contraction dim must sit on SBUF partitions):
  - x^T tiles   [d, s]    : xbar transpose-DMA from natural x
  - Q^T, K^T    [hk, s]   : direct result of projection matmuls (bf16)
  - V_aug       [skv, 65] : per skv-chunk, per head, bf16; col 64 = 1.0
  - S           [skv, sq] : PSUM f32; exp on ScalarE -> P bf16 in SBUF
  - vals^T      [hk, sq]  : bf16, A2A'd to sequence sharding
"""

import sys

sys.path.insert(0, "/opt/trn_rl_repo")

import ml_dtypes
import numpy as np

from concourse import bacc, bass, mybir, tile
from concourse.bass_utils import run_bass_kernel_spmd

S, D, H, K = 4096, 1024, 16, 64
NCORES = 8
HPC = H // NCORES          # heads per core (2)
HKC = HPC * K              # local head*dim columns (128)
SQ = S // NCORES           # seq rows owned per core (512)
SQT = 512                  # sq tile width in attention
NCH = S // 128             # total skv chunks (32)
F32 = mybir.dt.float32
BF16 = mybir.dt.bfloat16
EXP = mybir.ActivationFunctionType.Exp
BF16NP = ml_dtypes.bfloat16

_CACHE: dict = {}


def _build(causal: bool):
    nc = bacc.Bacc(
        "TRN2", target_bir_lowering=False, debug=False, num_devices=NCORES
    )
    cores = list(range(NCORES))

    x_full = nc.dram_tensor("x_full", [S, D], BF16, kind="ExternalInput")
    wq_c = nc.dram_tensor("wq_c", [D, HKC], BF16, kind="ExternalInput")
    wk_c = nc.dram_tensor("wk_c", [D, HKC], BF16, kind="ExternalInput")
    wv_c = nc.dram_tensor("wv_c", [D, HKC], BF16, kind="ExternalInput")
    wo_f = nc.dram_tensor("wo_f", [H * K, D], BF16, kind="ExternalInput")
    bq_c = nc.dram_tensor("bq_c", [HKC, 1], F32, kind="ExternalInput")
    bk_c = nc.dram_tensor("bk_c", [HKC, 1], F32, kind="ExternalInput")
    bv_c = nc.dram_tensor("bv_c", [HKC, 1], F32, kind="ExternalInput")
    bo_r = nc.dram_tensor("bo_r", [1, D], F32, kind="ExternalInput")
    masks01 = nc.dram_tensor("masks01", [128, 256], BF16, kind="ExternalInput")
    ident = nc.dram_tensor("ident", [128, 128], F32, kind="ExternalInput")
    out_t = nc.dram_tensor("out", [SQ, D], F32, kind="ExternalOutput")

    with tile.TileContext(nc) as tc:
        with tc.tile_pool(name="dram", bufs=1, space="DRAM") as dpool:
            a2a_in = dpool.tile([NCORES * HKC, SQ], BF16, name="a2a_in")
            a2a_out = dpool.tile([NCORES * HKC, SQ], BF16, name="a2a_out")

            with tc.tile_pool(name="persist", bufs=1) as pp:
                # ---- persistent SBUF ----
                dummy = pp.tile([128, SQT], BF16, name="dummy")
                nc.vector.memset(dummy, 0.5)
                wq_sb = pp.tile([128, 8, HKC], BF16, name="wq_sb")
                wk_sb = pp.tile([128, 8, HKC], BF16, name="wk_sb")
                wv_sb = pp.tile([128, 8, HKC], BF16, name="wv_sb")
                bq_sb = pp.tile([128, 1], F32, name="bq_sb")
                bk_sb = pp.tile([128, 1], F32, name="bk_sb")
                bv_sb = pp.tile([128, 1], F32, name="bv_sb")
                m01_sb = pp.tile([128, HPC, 128], BF16, name="m01_sb")
                ident_sb = pp.tile([128, 128], F32, name="ident_sb")
                nbias = pp.tile([128, 1], F32, name="nbias")
                nc.vector.memset(nbias, -3.0)
                qT_sb = pp.tile([128, S], BF16, name="qT_sb")
                kT_sb = pp.tile([128, S], BF16, name="kT_sb")
                v_aug = pp.tile([128, NCH, HPC, K + 1], BF16, name="v_aug")
                nc.vector.memset(v_aug, 1.0)  # presets the ones columns
                vals_sb = pp.tile([128, S], BF16, name="vals_sb")
                wo_sb = pp.tile([128, 8, D], BF16, name="wo_sb")
                bo_sb = pp.tile([1, D], F32, name="bo_sb")
                bo_bc = pp.tile([128, D], F32, name="bo_bc")

                with tc.tile_pool(name="xtp", bufs=3) as xtp, tc.tile_pool(
                    name="vtp", bufs=2
                ) as vtp, tc.tile_pool(
                    name="pj", bufs=2, space="PSUM"
                ) as pj, tc.tile_pool(
                    name="pS", bufs=2, space="PSUM"
                ) as pSp, tc.tile_pool(
                    name="pV0", bufs=1, space="PSUM"
                ) as pVp0, tc.tile_pool(
                    name="pV1", bufs=1, space="PSUM"
                ) as pVp1, tc.tile_pool(name="pT", bufs=6) as pTp, tc.tile_pool(
                    name="sm", bufs=2
                ) as smp:
                    pVp = (pVp0, pVp1)

                    # PE warmup: un-throttle the HAM clock gate before the
                    # first projection (needs ~3.4us of sustained PE busy).
                    for _ in range(8):
                        ps = pj.tile([128, SQT], F32, name="ps")
                        nc.tensor.matmul(
                            ps, lhsT=dummy[:, 0:128], rhs=dummy,
                            start=True, stop=True,
                        )
                    # preload the exp activation table set off the critical
                    # path (first real exp would otherwise pay ~2.7us)
                    wexp = smp.tile([128, 1], BF16, name="wexp")
                    nc.scalar.activation(
                        out=wexp, in_=dummy[:, 0:1], func=EXP,
                        scale=0.125, bias=nbias,
                    )

                    xts = {}

                    def issue_xt(t, split=False):
                        xt = xtp.tile([128, 8, SQT], BF16, name="xt")
                        src = x_full.ap()[t * SQT : (t + 1) * SQT, :]
                        if split:
                            # split the first transpose by d-chunk across
                            # both hwdge queues: proj(0)'s dc=0..3 matmuls
                            # can start as soon as the scalar-queue half
                            # lands
                            nc.scalar.dma_start_transpose(
                                out=xt[:, 0:4, :],
                                in_=src[:, 0 : D // 2],
                            )
                            nc.sync.dma_start_transpose(
                                out=xt[:, 4:8, :],
                                in_=src[:, D // 2 : D],
                            )
                        else:
                            nc.sync.dma_start_transpose(out=xt, in_=src)
                        xts[t] = xt

                    # first weight + bias on sync ahead of everything so
                    # proj(0) is never input-starved
                    nc.sync.dma_start(
                        out=wq_sb,
                        in_=wq_c.ap().rearrange("(a p) h -> p a h", p=128),
                    )
                    nc.sync.dma_start(out=bq_sb, in_=bq_c.ap())
                    issue_xt(0, split=True)
                    for wsb, wdr in ((wk_sb, wk_c), (wv_sb, wv_c)):
                        nc.sync.dma_start(
                            out=wsb,
                            in_=wdr.ap().rearrange("(a p) h -> p a h", p=128),
                        )
                    for bsb, bdr in ((bk_sb, bk_c), (bv_sb, bv_c)):
                        nc.sync.dma_start(out=bsb, in_=bdr.ap())
                    nc.sync.dma_start(
                        out=m01_sb, in_=masks01.ap().rearrange(
                            "p (h c) -> p h c", h=HPC
                        )
                    )
                    nc.sync.dma_start(out=ident_sb, in_=ident.ap())
                    issue_xt(1)

                    def proj_units(t):
                        """Projection for tile t as a list of small closures
                        sprinkled between attention chunk-steps: keeps
                        full-array matmuls flowing through the PE (HAM clock
                        gate stays warm) and hides proj work inside the
                        Scalar-paced attention pipeline."""
                        xt = xts.pop(t)
                        units = []

                        def mk_mms(wsb, box, dcs):
                            def go():
                                if box[0] is None:
                                    box[0] = pj.tile(
                                        [128, SQT], F32, name="ps"
                                    )
                                for dc in dcs:
                                    nc.tensor.matmul(
                                        box[0],
                                        lhsT=wsb[:, dc, :],
                                        rhs=xt[:, dc, :],
                                        start=(dc == 0),
                                        stop=(dc == 7),
                                    )
                            return go

                        def mk_bias(box, dst_sb, bias_sb):
                            def go():
                                nc.vector.tensor_scalar_add(
                                    out=dst_sb[:, t * SQT : (t + 1) * SQT],
                                    in0=box[0],
                                    scalar1=bias_sb,
                                )
                            return go

                        for which in range(3):
                            wsb = (wq_sb, wk_sb, wv_sb)[which]
                            box = [None]
                            for lo in (0, 4):
                                units.append(
                                    mk_mms(wsb, box, range(lo, lo + 4))
                                )
                            if which == 0:
                                units.append(mk_bias(box, qT_sb, bq_sb))
                            elif which == 1:
                                units.append(mk_bias(box, kT_sb, bk_sb))
                            else:
                                vb = [None]

                                def vbias(box=box, vb=vb):
                                    vb[0] = vtp.tile(
                                        [128, SQT], F32, name="vtt"
                                    )
                                    nc.vector.tensor_scalar_add(
                                        out=vb[0], in0=box[0], scalar1=bv_sb
                                    )
                                units.append(vbias)
                                # transpose V on the PE (v1 used the DMA
                                # xbar, which serialized on the hwdge queue)
                                pb = [None]

                                def mk_trans(js, vb=vb, pb=pb):
                                    def go():
                                        if pb[0] is None:
                                            pb[0] = pj.tile(
                                                [128, SQT], F32, name="ps"
                                            )
                                        for j in js:
                                            nc.tensor.transpose(
                                                pb[0][
                                                    :, j * 128 : (j + 1) * 128
                                                ],
                                                vb[0][
                                                    :, j * 128 : (j + 1) * 128
                                                ],
                                                ident_sb,
                                            )
                                    return go
                                units.append(mk_trans((0, 1)))
                                units.append(mk_trans((2, 3)))

                                def mk_copy(j, pb=pb):
                                    def go():
                                        for h in range(HPC):
                                            nc.vector.tensor_copy(
                                                out=v_aug[
                                                    :, t * 4 + j, h, 0:K
                                                ],
                                                in_=pb[0][
                                                    :,
                                                    j * 128 + h * K : j * 128
                                                    + (h + 1) * K,
                                                ],
                                            )
                                    return go
                                for j in range(4):
                                    units.append(mk_copy(j))
                        return units

                    def attn(t, pending):
                        nch = 4 * (t + 1) if causal else NCH
                        pv = {}
                        for h in range(HPC):
                            pv[h] = pVp[h].tile(
                                [K + 1, SQT], F32, name=f"pv{h}"
                            )
                        def s_and_exp(ch):
                            jm = ch - 4 * t
                            off = jm * 128 if (causal and 0 < jm < 4) else 0
                            # both heads share one PSUM tile (adjacent
                            # banks) so exp covers both in ONE ScalarE
                            # instruction (each instr pays ~293ns fixed)
                            pS = pSp.tile([128, HPC, SQT], F32, name="pS")
                            # paired score matmuls, back-to-back: head0 in
                            # PE rows 0:64, head1 in rows 64:128 -> they
                            # run concurrently in distinct row groups
                            for h in range(HPC):
                                hs = h * K
                                nc.tensor.matmul(
                                    pS[:, h, off:SQT],
                                    lhsT=kT_sb[
                                        hs : hs + K,
                                        ch * 128 : (ch + 1) * 128,
                                    ],
                                    rhs=qT_sb[
                                        hs : hs + K,
                                        t * SQT + off : (t + 1) * SQT,
                                    ],
                                    start=True,
                                    stop=True,
                                )
                            pT = pTp.tile([128, HPC, SQT], BF16, name="pT")
                            nc.scalar.activation(
                                out=pT[:, :, off:SQT],
                                in_=pS[:, :, off:SQT],
                                func=EXP,
                                scale=0.125,
                                bias=nbias,
                            )
                            if causal and 0 <= jm < 4:
                                # causal mask applied AFTER exp by zeroing
                                # the diagonal block of pT (0/1 multiply on
                                # Vector) — keeps ScalarE's input chain
                                # free of Vector ops; exp of unmasked
                                # scores stays finite (|s|/8 - 3 < 3)
                                bo_ = jm * 128
                                nc.vector.tensor_mul(
                                    out=pT[:, :, bo_ : bo_ + 128],
                                    in0=pT[:, :, bo_ : bo_ + 128],
                                    in1=m01_sb,
                                )
                            return pT, off

                        def av(ch, pT, off):
                            for h in range(HPC):
                                nc.tensor.matmul(
                                    pv[h][:, off:SQT],
                                    lhsT=v_aug[:, ch, h, :],
                                    rhs=pT[:, h, off:SQT],
                                    start=(ch == 0),
                                    stop=(ch == nch - 1),
                                )

                        # software pipeline: scores/exp of step ch+1 are
                        # emitted BEFORE AV of step ch, so the in-order PE
                        # never sits directly behind the exp it feeds
                        prev = None
                        for ch in range(nch):
                            cur = (ch, *s_and_exp(ch))
                            if prev is not None:
                                av(*prev)
                            # sprinkle next tile's projection between
                            # chunk-steps (runs while exp(ch) is on ScalarE)
                            nun = len(pending)
                            if nun and ch < nch - 1:
                                k = max(1, -(-nun // (nch - 1 - ch)))
                                for _ in range(min(k, nun)):
                                    pending.pop(0)()
                            prev = cur
                        av(*prev)
                        while pending:
                            pending.pop(0)()
                        # copy AV numerators + denominator rows out of PSUM
                        # immediately (frees the pv banks for the next
                        # tile); the divide itself is deferred into the
                        # next tile where the slow [2,512] reciprocal hides
                        # under exp work
                        cval0 = smp.tile([K, SQT], F32, name="cval0")
                        cval1 = smp.tile([K, SQT], F32, name="cval1")
                        dn0 = smp.tile([1, SQT], F32, name="dn0")
                        dn1 = smp.tile([1, SQT], F32, name="dn1")
                        for h, cv, dnt in ((0, cval0, dn0), (1, cval1, dn1)):
                            nc.vector.tensor_copy(out=cv, in_=pv[h][0:K, :])
                            nc.vector.tensor_copy(
                                out=dnt, in_=pv[h][K : K + 1, :]
                            )

                        def divwork():
                            # approx reciprocal (~4e-6 rel, 5x faster than
                            # the exact InstReciprocal; denominators are
                            # sums of exps, strictly positive)
                            recip0 = smp.tile([1, SQT], F32, name="recip0")
                            recip1 = smp.tile([1, SQT], F32, name="recip1")
                            nc.vector.reciprocal_approx_fast(
                                out=recip0, in_=dn0
                            )
                            nc.vector.reciprocal_approx_fast(
                                out=recip1, in_=dn1
                            )
                            bcn0 = smp.tile([K, SQT], F32, name="bcn0")
                            bcn1 = smp.tile([K, SQT], F32, name="bcn1")
                            nc.gpsimd.partition_broadcast(bcn0, recip0)
                            nc.gpsimd.partition_broadcast(bcn1, recip1)
                            for h, cv, bcn in ((0, cval0, bcn0), (1, cval1, bcn1)):
                                nc.vector.tensor_mul(
                                    out=vals_sb[
                                        h * K : (h + 1) * K,
                                        t * SQT : (t + 1) * SQT,
                                    ],
                                    in0=cv,
                                    in1=bcn,
                                )
                            nc.sync.dma_start(
                                out=a2a_in[t * HKC : (t + 1) * HKC, :],
                                in_=vals_sb[:, t * SQT : (t + 1) * SQT],
                            )
                        return divwork

                    # proj(0) runs up front (nothing to interleave with);
                    # proj(t+1) is sprinkled through attn(t)'s chunk-steps
                    for u in proj_units(0):
                        u()
                    divdefer = None
                    for t in range(8):
                        if t == 0:
                            # xt(2) transpose on the scalar hwdge queue:
                            # it executes during ScalarE's early idle
                            # window (attn(0) has only 4 exp steps)
                            xt2 = xtp.tile([128, 8, SQT], BF16, name="xt")
                            nc.scalar.dma_start_transpose(
                                out=xt2,
                                in_=x_full.ap()[2 * SQT : 3 * SQT, :],
                            )
                            xts[2] = xt2
                        elif t + 2 < 8:
                            issue_xt(t + 2)
                        if t == 1:
                            # big late loads, hidden behind attention
                            nc.sync.dma_start(
                                out=wo_sb,
                                in_=wo_f.ap().rearrange(
                                    "(a p) d -> p a d", p=128
                                ),
                            )
                            nc.sync.dma_start(out=bo_sb, in_=bo_r.ap())
                            nc.gpsimd.partition_broadcast(bo_bc, bo_sb)
                        pending = [divdefer] if divdefer else []
                        pending += proj_units(t + 1) if t + 1 < 8 else []
                        divdefer = attn(t, pending)
                    divdefer()

                nc.gpsimd.collective_compute(
                    "AllToAll",
                    mybir.AluOpType.bypass,
                    replica_groups=[cores],
                    ins=[a2a_in.opt()],
                    outs=[a2a_out.opt()],
                )

                # ---- output projection (sequence-sharded) ----
                with tc.tile_pool(name="op", bufs=1) as op, tc.tile_pool(
                    name="po", bufs=8, space="PSUM"
                ) as pop:
                    va_sb = op.tile([128, 8, SQT], BF16, name="va_sb")
                    a2a_or = a2a_out.rearrange("(a p) s -> p a s", p=128)
                    for hkc in range(8):
                        # per-chunk loads so the first matmuls start as
                        # soon as the first 128 hk rows land
                        nc.sync.dma_start(
                            out=va_sb[:, hkc, :], in_=a2a_or[:, hkc, :]
                        )
                    o_sb = op.tile([128, 4, D], F32, name="o_sb")
                    pos = {}
                    for m in range(4):
                        for dh in range(2):
                            pos[m, dh] = pop.tile([128, 512], F32, name="po")
                    for hkc in range(8):
                        for m in range(4):
                            for dh in range(2):
                                nc.tensor.matmul(
                                    pos[m, dh],
                                    lhsT=va_sb[:, hkc, m * 128 : (m + 1) * 128],
                                    rhs=wo_sb[:, hkc, dh * 512 : (dh + 1) * 512],
                                    start=(hkc == 0),
                                    stop=(hkc == 7),
                                )
                    for m in range(4):
                        for dh in range(2):
                            nc.vector.tensor_add(
                                out=o_sb[:, m, dh * 512 : (dh + 1) * 512],
                                in0=pos[m, dh],
                                in1=bo_bc[:, dh * 512 : (dh + 1) * 512],
                            )
                        nc.sync.dma_start(
                            out=out_t.ap()[m * 128 : (m + 1) * 128, :],
                            in_=o_sb[:, m, :],
                        )

    nc.compile()
    return nc


def _get_nc(causal: bool):
    if causal not in _CACHE:
        _CACHE[causal] = _build(causal)
    return _CACHE[causal]


def _make_in_maps(x, wq, bq, wk, bk, wv, bv, wo, bo):
    x = np.ascontiguousarray(
        np.asarray(x, np.float32).reshape(S, D).astype(BF16NP)
    )
    wqf = np.asarray(wq, np.float32).reshape(D, H * K).astype(BF16NP)
    wkf = np.asarray(wk, np.float32).reshape(D, H * K).astype(BF16NP)
    wvf = np.asarray(wv, np.float32).reshape(D, H * K).astype(BF16NP)
    wof = np.ascontiguousarray(
        np.asarray(wo, np.float32).reshape(H * K, D).astype(BF16NP)
    )
    bqf = np.asarray(bq, np.float32).reshape(H * K)
    bkf = np.asarray(bk, np.float32).reshape(H * K)
    bvf = np.asarray(bv, np.float32).reshape(H * K)
    bof = np.ascontiguousarray(np.asarray(bo, np.float32).reshape(1, D))

    p = np.arange(128)[:, None]
    c = np.arange(128)[None, :]
    m01 = np.where(c >= p, 1.0, 0.0).astype(np.float32)
    masks01_np = np.ascontiguousarray(
        np.concatenate([m01, m01], axis=1)
    ).astype(BF16NP)
    ident_np = np.eye(128, dtype=np.float32)

    in_maps = []
    for core in range(NCORES):
        hk0 = core * HKC
        in_maps.append(
            {
                "x_full": x,
                "wq_c": np.ascontiguousarray(wqf[:, hk0 : hk0 + HKC]),
                "wk_c": np.ascontiguousarray(wkf[:, hk0 : hk0 + HKC]),
                "wv_c": np.ascontiguousarray(wvf[:, hk0 : hk0 + HKC]),
                "wo_f": wof,
                "bq_c": np.ascontiguousarray(
                    bqf[hk0 : hk0 + HKC].reshape(HKC, 1)
                ),
                "bk_c": np.ascontiguousarray(
                    bkf[hk0 : hk0 + HKC].reshape(HKC, 1)
                ),
                "bv_c": np.ascontiguousarray(
                    bvf[hk0 : hk0 + HKC].reshape(HKC, 1)
                ),
                "bo_r": bof,
                "masks01": masks01_np,
                "ident": ident_np,
            }
        )
    return in_maps


def _run(inputs: dict, trace: bool = False):
    causal = bool(int(np.asarray(inputs["is_causal"])))
    nc = _get_nc(causal)
    in_maps = _make_in_maps(
        inputs["x"], inputs["wq"], inputs["bq"], inputs["wk"], inputs["bk"],
        inputs["wv"], inputs["bv"], inputs["wo"], inputs["bo"],
    )
    res = run_bass_kernel_spmd(
        nc, in_maps, list(range(NCORES)), trace=trace
    )
    out = np.empty((1, S, D), np.float32)
    for core in range(NCORES):
        out[0, core * SQ : (core + 1) * SQ] = res.results[core]["out"]
    return out, res


def kernel(**inputs) -> np.ndarray:
    out, _ = _run(inputs, trace=False)
    return out
